# revision 36
# baseline (speedup 1.0000x reference)
"""AdaLN self-attention block (B=2, L=2048, C=1024, H=16, DFF=4096) on 8 TRN2 cores.

Sharding: DP=2 over batch (cores 0-3 -> batch 0, cores 4-7 -> batch 1),
sequence-parallel 4-way within each group (512 query tokens per core).
Each core holds full weights (bf16), computes q/k/v for its own 512 tokens,
all-gathers normalized K and V (with an appended ones column for the softmax
denominator) within its 4-core group, runs full attention for its queries,
then proj + FFN locally on its token slice. Host concatenates the slices.

Everything on-chip is feature-major ([C, tokens]); the host pre-transposes
activations/weights so no on-device transposes are needed (except a tiny
48x128 one for the adaLN modulation vector).
"""

import os
import sys

for _p in ("/opt/trn_rl_repo", os.path.expanduser("~/.axon_site/_ro/trn_rl_repo")):
    if os.path.isdir(_p) and _p not in sys.path:
        sys.path.insert(0, _p)

import numpy as np
import ml_dtypes

import concourse.bass as bass
import concourse.tile as tile
from concourse import mybir
from concourse.bass import ds, ts
from concourse import bass_utils

BF16 = mybir.dt.bfloat16
F32 = mybir.dt.float32
AF = mybir.ActivationFunctionType

B, L, C, H, DH, DFF, D = 2, 2048, 1024, 16, 64, 4096, 1024
NCORES = 8
GROUP = 4          # cores per batch group
T = L // GROUP     # 512 query tokens per core
CT = C // 128      # 8 feature tiles
ADA_SLICE = 6 * C // GROUP  # 1536 adaLN outputs per core
EPS = 1e-6
MAX_SCALE_MUL = float(np.log(100.0))
REPLICA_GROUPS = [[0, 1, 2, 3], [4, 5, 6, 7]]

_CACHE = {}


# --------------------------------------------------------------------------- #
# graph construction
# --------------------------------------------------------------------------- #

def _build(sim_gelu=False, split_waits=True):
    nc = bass.Bass(
        "TRN2", target_bir_lowering=False, debug=False, num_devices=NCORES
    )

    def inp(name, shape, dt):
        return nc.dram_tensor(name, shape, dt, kind="ExternalInput").ap()

    p = {
        "xb": inp("xb", [CT, 128, T], BF16),      # x^T slice, tiled, bf16
        "cond8": inp("cond8", [128, 8], F32),     # cond feature-major
        "biasT": inp("biasT", [16, 128, T], BF16),  # exp-bias source, tiled
        "qkwT": inp("qkwT", [CT, 4, 128, 512], BF16),
        "vwT": inp("vwT", [CT, 128, 1024], BF16),
        "projwT": inp("projwT", [CT, 2, 128, 512], BF16),
        "fc1wT": inp("fc1wT", [CT, 8, 128, 512], BF16),
        "fc2wT": inp("fc2wT", [DFF // 128, 2, 128, 512], BF16),
        "adawT": inp("adawT", [CT, 3, 128, 512], BF16),
        "adab48": inp("adab48", [128, 48], F32),
        "qb8": inp("qb8", [128, CT], F32),
        "vb2": inp("vb2", [1, C], BF16),
        "pb8": inp("pb8", [128, CT], F32),
        "f1b": inp("f1b", [128, DFF // 128], F32),
        "f2b": inp("f2b", [128, CT], F32),
        "smv": inp("smv", [16, 1], F32),
        "ones128": inp("ones128", [128, 128], BF16),
        "hsel": inp("hsel", [128, CT, 16], BF16),
        "ones1_128": inp("ones1_128", [1, 128], BF16),
        "pairsel": inp("pairsel", [2, 128], BF16),
        "ones1_64": inp("ones1_64", [1, 64], BF16),
        "eye48": inp("eye48", [48, 48], F32),
    }
    out = nc.dram_tensor("out", [C, T], F32, kind="ExternalOutput").ap()

    with tile.TileContext(nc) as tc:
        _emit(nc, tc, p, out, sim_gelu)
    if split_waits:
        _split_waits(nc)
    return nc


_SPLIT_TYPES = {
    "InstTensorTensor", "InstTensorScalarPtr", "InstReciprocal",
    "InstTensorCopy", "InstActivation", "InstTensorReduce", "InstMemset",
    "InstMatmult", "InstLdweights", "InstCopyPredicated", "InstBnStats",
    "InstBnAggr", "InstStreamTranspose", "InstDMACopy", "InstDrain",
    "InstCollectiveCompute",
}


def _split_waits(nc, max_waits=1):
    """Walrus TPB codegen rejects >1 sync-wait on compute instructions;
    hoist extras onto standalone EventSemaphore waits on the same engine."""
    for bb in nc.main_func.blocks:
        new = []
        changed = False
        for ins in bb.instructions:
            si = getattr(ins, "sync_info", None)
            if (
                si is not None
                and si.on_wait
                and len(si.on_wait) > max_waits
                and type(ins).__name__ in _SPLIT_TYPES
            ):
                waits = list(si.on_wait)
                for i, w in enumerate(waits[:-max_waits]):
                    ws = mybir.InstEventSemaphore(
                        name=f"{ins.name}_w{i}", ins=[], outs=[]
                    )
                    ws.engine = ins.engine
                    ws.sync_info = mybir.SyncInfo(on_wait=[w], on_update=[])
                    new.append(ws)
                ins.sync_info = mybir.SyncInfo(
                    on_wait=waits[-max_waits:], on_update=list(si.on_update)
                )
                changed = True
            new.append(ins)
        if changed:
            bb.instructions = new


def _bcast_rows(row_ap, parts):
    """[1, N] AP -> [parts, N] AP with partition step 0 (DMA broadcast read)."""
    return bass.AP(tensor=row_ap.tensor, offset=row_ap.offset,
                   ap=[[0, parts]] + list(row_ap.ap[1:]))


def _emit(nc, tc, p, out_d, sim_gelu=False):

    # ---- persistent SBUF pools -------------------------------------------- #
    const = tc.alloc_tile_pool(name="const", bufs=1)
    persist = tc.alloc_tile_pool(name="persist", bufs=1)
    work = tc.alloc_tile_pool(name="work", bufs=3)
    stats = tc.alloc_tile_pool(name="stats", bufs=1)
    wpool = tc.alloc_tile_pool(name="wpool", bufs=8)
    dram = tc.alloc_tile_pool(name="dram", bufs=1, space="DRAM")

    # ---- constants / small inputs to SBUF --------------------------------- #
    def load_const(name, shape, dt):
        t = const.tile(shape, dt, tag=name)
        nc.sync.dma_start(out=t[:], in_=p[name])
        return t

    ones128 = load_const("ones128", [128, 128], BF16)
    hsel = load_const("hsel", [128, CT, 16], BF16)
    ones1_128 = load_const("ones1_128", [1, 128], BF16)
    pairsel = load_const("pairsel", [2, 128], BF16)
    ones1_64 = load_const("ones1_64", [1, 64], BF16)
    eye48 = load_const("eye48", [48, 48], F32)
    qb8 = load_const("qb8", [128, CT], F32)
    vb2 = load_const("vb2", [1, C], BF16)
    pb8 = load_const("pb8", [128, CT], F32)
    f1b = load_const("f1b", [128, DFF // 128], F32)
    f2b = load_const("f2b", [128, CT], F32)
    smv_in = load_const("smv", [16, 1], F32)
    cond8 = load_const("cond8", [128, 8], F32)
    adab48 = load_const("adab48", [128, 48], F32)

    # ---- persistent activations ------------------------------------------- #
    xb = persist.tile([128, CT, T], BF16, tag="big_d")       # x^T bf16
    nc.sync.dma_start(out=xb[:], in_=p["xb"].rearrange("t p q -> p t q"))

    h1 = persist.tile([128, CT, T], BF16, tag="big_a")       # LN1-modulated
    qe = persist.tile([128, CT, T], BF16, tag="big_b")       # q (later normed)
    ke = persist.tile([128, CT, T], BF16, tag="big_c")       # k (later normed)
    v_pre = persist.tile([128, T // 128, H, DH + 1], BF16, tag="vpre")
    eb = persist.tile([128, L // 128, T], BF16, tag="eb")    # exp(bias^T)
    attn = persist.tile([128, CT, T], BF16, tag="big_a")
    x2 = persist.tile([128, CT, T], BF16, tag="big_c")
    h2 = persist.tile([128, CT, T], BF16, tag="big_b")
    gact = persist.tile([128, DFF // 128, T], BF16, tag="big_d")
    kfull = persist.tile([128, CT, GROUP, T], BF16, tag="kfull")
    mod = persist.tile([128, 48], F32, tag="mod")

    # ---- DRAM bounce buffers ---------------------------------------------- #
    ada_in = dram.tile([1, ADA_SLICE], F32, tag="ada_in")
    ada_g = dram.tile([GROUP, ADA_SLICE], F32, tag="ada_g")
    k_in = dram.tile([C, T], BF16, tag="k_in")
    k_g = dram.tile([GROUP, C, T], BF16, tag="k_g")
    v_in = dram.tile([T, H, DH + 1], BF16, tag="v_in")
    v_g = dram.tile([GROUP, T, H, DH + 1], BF16, tag="v_g")
    rq_d = dram.tile([16, T], BF16, tag="rq_d")
    rk_d = dram.tile([16, T], BF16, tag="rk_d")

    # ---- expbias (independent; emitted early so it overlaps) -------------- #
    nc.sync.dma_start(out=eb[:], in_=p["biasT"].rearrange("t p q -> p t q"))
    for i in range(4):
        nc.scalar.activation(
            out=eb[:, ds(4 * i, 4), :], in_=eb[:, ds(4 * i, 4), :], func=AF.Exp
        )

    # ---- scale_mul -> smv = exp(min(scale_mul, log 100)) ------------------ #
    eps128 = const.tile([128, 1], F32, tag="eps128")
    nc.vector.memset(eps128[:], EPS)
    smv = stats.tile([16, 1], F32, tag="smv")
    nc.vector.tensor_scalar_min(out=smv[:], in0=smv_in[:], scalar1=MAX_SCALE_MUL)
    nc.scalar.activation(out=smv[:], in_=smv[:], func=AF.Exp)

    # ============================ phase 1 PSUM ============================= #
    ps1 = tc.alloc_tile_pool(name="ps1", bufs=1, space="PSUM")

    # ---- adaLN: silu(cond) @ ada_w^T slice, then group all-gather --------- #
    sig = work.tile([128, 8], F32, tag="w8")
    nc.scalar.activation(out=sig[:], in_=cond8[:], func=AF.Exp, scale=-1.0)
    nc.vector.tensor_scalar_add(out=sig[:], in0=sig[:], scalar1=1.0)
    nc.vector.reciprocal(out=sig[:], in_=sig[:])
    silu = work.tile([128, 8], BF16, tag="w8b")
    nc.vector.tensor_tensor(
        out=silu[:], in0=sig[:], in1=cond8[:], op=mybir.AluOpType.mult
    )

    _sc_ada = nc.named_scope("ada"); _sc_ada.__enter__()
    for n in range(3):
        aps = ps1.tile([1, 512], F32, tag="sm", bufs=2, name="aps")
        for k in range(CT):
            wt = wpool.tile([128, 512], BF16, tag="w512", name="wada")
            nc.sync.dma_start(out=wt[:], in_=p["adawT"][k, n])
            nc.tensor.matmul(
                aps[:], silu[:, ds(k, 1)], wt[:], start=(k == 0), stop=(k == CT - 1)
            )
        aw = work.tile([1, 512], F32, tag="w1x512", name="aw")
        nc.vector.tensor_copy(out=aw[:], in_=aps[:])
        nc.sync.dma_start(out=ada_in[0, ds(512 * n, 512)], in_=aw[:])
    nc.gpsimd.collective_compute(
        "AllGather", mybir.AluOpType.bypass, replica_groups=REPLICA_GROUPS,
        ins=[ada_in.opt()], outs=[ada_g.opt()],
    )
    # load [48,128] token-major, transpose on PE -> mod [128, 48]
    ada_tm = work.tile([48, 128], F32, tag="ada_tm")
    nc.sync.dma_start(out=ada_tm[:], in_=ada_g.rearrange("g n -> (g n)").rearrange("(j p) -> j p", p=128))
    modps = ps1.tile([128, 48], F32, tag="sm", bufs=2)
    nc.tensor.transpose(modps[:], ada_tm[:], eye48[:])
    nc.vector.tensor_tensor(out=mod[:], in0=modps[:], in1=adab48[:],
                            op=mybir.AluOpType.add)
    # s1, s2 chunks get +1
    nc.vector.tensor_scalar_add(out=mod[:, 16:32], in0=mod[:, 16:32], scalar1=1.0)
    _sc_ada.__exit__(None, None, None)

    # ---- layernorm helper (feature-major, partition sums via ones matmul) - #
    def layernorm(src, dst, s_col, sh_col, psp):
        s1 = psp.tile([128, T], F32, tag="mm", bufs=4 if psp.name == "ps1" else 6)
        s2 = psp.tile([128, T], F32, tag="mm", bufs=4 if psp.name == "ps1" else 6)
        for t in range(CT):
            sq = work.tile([128, T], BF16, tag="sq")
            nc.vector.tensor_tensor(
                out=sq[:], in0=src[:, t, :], in1=src[:, t, :], op=mybir.AluOpType.mult
            )
            nc.tensor.matmul(s1[:], ones128[:], src[:, t, :],
                             start=(t == 0), stop=(t == CT - 1), skip_group_check=True)
            nc.tensor.matmul(s2[:], ones128[:], sq[:],
                             start=(t == 0), stop=(t == CT - 1), skip_group_check=True)
        meanb = stats.tile([128, T], F32, tag="meanb")
        nc.vector.tensor_scalar_mul(out=meanb[:], in0=s1[:], scalar1=1.0 / C)
        m2 = stats.tile([128, T], F32, tag="m2")
        nc.vector.tensor_tensor(out=m2[:], in0=meanb[:], in1=meanb[:],
                                op=mybir.AluOpType.mult)
        varb = stats.tile([128, T], F32, tag="varb")
        nc.vector.scalar_tensor_tensor(
            out=varb[:], in0=s2[:], scalar=1.0 / C, in1=m2[:],
            op0=mybir.AluOpType.mult, op1=mybir.AluOpType.subtract,
        )
        # rstd = exp(-0.5 * ln(var + eps))   (stays in the exp/ln table set)
        nc.scalar.activation(out=varb[:], in_=varb[:], func=AF.Ln, bias=eps128[:])
        rstdb = stats.tile([128, T], F32, tag="rstdb")
        nc.scalar.activation(out=rstdb[:], in_=varb[:], func=AF.Exp, scale=-0.5)
        for t in range(CT):
            d1 = work.tile([128, T], F32, tag="d1")
            nc.vector.tensor_tensor(out=d1[:], in0=src[:, t, :], in1=meanb[:],
                                    op=mybir.AluOpType.subtract)
            nc.vector.tensor_tensor(out=d1[:], in0=d1[:], in1=rstdb[:],
                                    op=mybir.AluOpType.mult)
            nc.vector.tensor_scalar(
                out=dst[:, t, :], in0=d1[:],
                scalar1=mod[:, ds(s_col + t, 1)], scalar2=mod[:, ds(sh_col + t, 1)],
                op0=mybir.AluOpType.mult, op1=mybir.AluOpType.add,
            )

    with nc.named_scope("ln1"):
        layernorm(xb, h1, 16, 32, ps1)  # s1 cols 16..23, sh1 cols 32..39

    # ---- qkv: K first (so its all-gather overlaps V and Q compute) ------- #
    _sc_qkv = nc.named_scope("qkv"); _sc_qkv.__enter__()
    ssq_k = ps1.tile([16, T], F32, tag="ss", bufs=2)

    def qk_block(mgs, is_q, ssq=None):
        for mg in mgs:
            accs = [ps1.tile([128, T], F32, tag="mm", bufs=4, name=f"qk{mg}_{i}")
                    for i in range(4)]
            for k in range(CT):
                wt = wpool.tile([128, 4, 128], BF16, tag="w512", name="wqk")
                nc.sync.dma_start(
                    out=wt[:], in_=p["qkwT"][k, mg].rearrange("p (j s) -> p j s", j=4)
                )
                for mj in range(4):
                    nc.tensor.matmul(accs[mj][:], wt[:, mj, :], h1[:, k, :],
                                     start=(k == 0), stop=(k == CT - 1))
            for mj in range(4):
                m = 4 * mg + mj
                acc = accs[mj]
                if is_q:
                    dst = qe[:, m, :]
                    nc.vector.tensor_scalar_add(out=dst, in0=acc[:],
                                                scalar1=qb8[:, ds(m, 1)])
                else:
                    dst = ke[:, m - 8, :]
                    nc.vector.tensor_copy(out=dst, in_=acc[:])
                sq = work.tile([128, T], BF16, tag="sq")
                nc.vector.tensor_tensor(out=sq[:], in0=dst, in1=dst,
                                        op=mybir.AluOpType.mult)
                tgt = ssq[0] if is_q else ssq_k
                tm = m % 8
                nc.tensor.matmul(tgt[:], hsel[:, tm, :], sq[:],
                                 start=(tm == 0), stop=(tm == 7),
                                 skip_group_check=True)

    def make_rnorm(ssq, with_sm):
        r = stats.tile([16, T], F32, tag="rn_f")
        nc.vector.tensor_scalar_max(out=r[:], in0=ssq[:], scalar1=1e-24)
        nc.scalar.activation(out=r[:], in_=r[:], func=AF.Ln)
        rb = stats.tile([16, T], BF16, tag="rn_bq" if with_sm else "rn_bk", name="rb")
        nc.scalar.activation(out=rb[:], in_=r[:], func=AF.Exp, scale=-0.5)
        if with_sm:
            nc.vector.tensor_scalar_mul(out=rb[:], in0=rb[:], scalar1=smv[:])
        return rb

    def rnorm_apply(rb, rd_bounce, dst, psp):
        # partition remap [16,T] -> [2,8,T] via a DRAM roundtrip, then a
        # K=2 pairsel matmul broadcasts each head row over its 64 partitions
        nc.sync.dma_start(out=rd_bounce[:], in_=rb[:])
        rn2 = work.tile([2, 8, T], BF16, tag="rn2", bufs=1, name="rn2")
        nc.sync.dma_start(out=rn2[:],
                          in_=rd_bounce.rearrange("(t j) q -> j t q", j=2))
        for t in range(CT):
            bc = psp.tile([128, T], F32, tag="sm", bufs=2, name="bcn")
            nc.tensor.matmul(bc[:], pairsel[:], rn2[:, t, :], start=True, stop=True)
            nc.vector.tensor_tensor(out=dst[:, t, :], in0=dst[:, t, :], in1=bc[:],
                                    op=mybir.AluOpType.mult)

    qk_block((2, 3), False)  # K tiles
    rkn = make_rnorm(ssq_k, False)
    rnorm_apply(rkn, rk_d, ke, ps1)
    _sc_qkv.__exit__(None, None, None)

    nc.sync.dma_start(out=k_in.rearrange("(t p) q -> p t q", p=128), in_=ke[:])
    with nc.named_scope("agK"):
        nc.gpsimd.collective_compute(
            "AllGather", mybir.AluOpType.bypass, replica_groups=REPLICA_GROUPS,
            ins=[k_in.opt()], outs=[k_g.opt()],
        )
    for t in range(CT):
        nc.sync.dma_start(
            out=kfull[:, t, :, :],
            in_=k_g.rearrange("r (t p) q -> p t r q", p=128)[:, t, :, :],
        )
    # ---- V (token-major) + ones column, then all-gather ------------------- #
    _sc_v = nc.named_scope("vphase"); _sc_v.__enter__()
    nc.vector.memset(v_pre[:, :, :, DH : DH + 1], 1.0)
    for tcn in range(T // 128):
        accs = [ps1.tile([128, 512], F32, tag="mm", bufs=4, name=f"vacc{tcn}_{i}") for i in range(2)]
        for k in range(CT):
            wt = wpool.tile([128, 1024], BF16, tag="w1024", bufs=4)
            nc.sync.dma_start(out=wt[:], in_=p["vwT"][k])
            for vf in range(2):
                nc.tensor.matmul(
                    accs[vf][:], h1[:, k, ds(128 * tcn, 128)],
                    wt[:, ds(512 * vf, 512)],
                    start=(k == 0), stop=False, skip_group_check=True,
                )
        for vf in range(2):
            nc.tensor.matmul(
                accs[vf][:], ones1_128[:], vb2[:, ds(512 * vf, 512)],
                start=False, stop=True, skip_group_check=True,
            )
            nc.vector.tensor_copy(
                out=v_pre[:, tcn, ds(8 * vf, 8), 0:DH],
                in_=accs[vf][:].rearrange("p (h d) -> p h d", d=DH),
            )
    nc.sync.dma_start(
        out=v_in.rearrange("(tc p) h d -> p tc h d", p=128), in_=v_pre[:]
    )
    _sc_v.__exit__(None, None, None)
    with nc.named_scope("agV"):
        nc.gpsimd.collective_compute(
            "AllGather", mybir.AluOpType.bypass, replica_groups=REPLICA_GROUPS,
            ins=[v_in.opt()], outs=[v_g.opt()],
        )

    # ---- Q tiles last (overlap the gathers) ------------------------------- #
    _sc_q = nc.named_scope("qtiles"); _sc_q.__enter__()
    ssq_q = ps1.tile([16, T], F32, tag="ss", bufs=2)
    qk_block((0, 1), True, ssq=[ssq_q])
    rqn = make_rnorm(ssq_q, True)
    rnorm_apply(rqn, rq_d, qe, ps1)
    _sc_q.__exit__(None, None, None)

    ps1.release()

    # ============================ attention ================================ #
    ps2 = tc.alloc_tile_pool(name="ps2", bufs=1, space="PSUM")
    vpool = tc.alloc_tile_pool(name="vpool", bufs=2)
    ptpool = tc.alloc_tile_pool(name="ptpool", bufs=5)

    KT = L // 128  # 16 key tiles
    _sc_at = nc.named_scope("attn"); _sc_at.__enter__()
    for pr in range(8):  # head pairs
        vt = vpool.tile([128, KT, 2, DH + 1], BF16, tag="v")
        for r in range(GROUP):
            nc.sync.dma_start(
                out=vt[:, ds(4 * r, 4), :, :],
                in_=v_g[r].rearrange(
                    "(kt p) h d -> p kt h d", p=128
                )[:, :, ds(2 * pr, 2), :],
            )
        vsb = [vt[:, :, 0, :], vt[:, :, 1, :]]
        o_ps = [ps2.tile([DH + 1, T], F32, tag="o", bufs=4, name=f"o{pr}_{i}") for i in range(2)]
        for kt in range(KT):
            r, ktc = divmod(kt, 4)
            sp = ps2.tile([128, 2 * T], F32, tag="sc", bufs=2)
            for j in range(2):
                nc.tensor.matmul(
                    sp[:, ds(T * j, T)],
                    kfull[ds(64 * j, 64), pr, r, ds(128 * ktc, 128)],
                    qe[ds(64 * j, 64), pr, :],
                    start=True, stop=True,
                )
            pt = ptpool.tile([128, 2 * T], BF16, tag="pt")
            nc.scalar.activation(out=pt[:], in_=sp[:], func=AF.Exp)
            ebs = eb[:, kt, :]
            eb2 = bass.AP(tensor=ebs.tensor, offset=ebs.offset,
                          ap=[list(ebs.ap[0]), [0, 2], list(ebs.ap[1])])
            nc.vector.tensor_tensor(out=pt[:], in0=pt[:], in1=eb2,
                                    op=mybir.AluOpType.mult)
            for j in range(2):
                nc.tensor.matmul(
                    o_ps[j][:], vsb[j][:, kt, :], pt[:, ds(T * j, T)],
                    start=(kt == 0), stop=(kt == KT - 1), skip_group_check=True,
                )
        for j in range(2):
            rd = work.tile([1, T], BF16, tag="rd")
            with nc.allow_low_precision(reason="softmax denom recip in bf16 is fine at 2e-2 tol"):
                nc.vector.reciprocal(out=rd[:], in_=o_ps[j][ds(DH, 1), :])
            oc = work.tile([64, T], BF16, tag="bcs", name="oc")
            nc.vector.tensor_copy(out=oc[:], in_=o_ps[j][0:DH, :])
            bcp = ps2.tile([64, T], F32, tag="o", bufs=4, name="bcp")
            nc.tensor.matmul(bcp[:], ones1_64[:], rd[:], start=True, stop=True)
            nc.vector.tensor_tensor(
                out=attn[ds(64 * j, 64), pr, :], in0=oc[:], in1=bcp[:],
                op=mybir.AluOpType.mult,
            )

    _sc_at.__exit__(None, None, None)
    ptpool.release()
    vpool.release()
    ps2.release()

    # ============================ proj + FFN =============================== #
    ps3 = tc.alloc_tile_pool(name="ps3", bufs=1, space="PSUM")

    pbg1 = stats.tile([128, CT], F32, tag="pbg1")
    nc.vector.tensor_tensor(out=pbg1[:], in0=pb8[:], in1=mod[:, 0:8],
                            op=mybir.AluOpType.mult)
    fbg2 = stats.tile([128, CT], F32, tag="fbg2")
    nc.vector.tensor_tensor(out=fbg2[:], in0=f2b[:], in1=mod[:, 8:16],
                            op=mybir.AluOpType.mult)

    def dense(wT, K, Mtiles, rhs, consume):
        """out[m] = sum_k wT[k,:].T[128m slice] @ rhs[:,k,:]  (feature-major)."""
        for mg in range((Mtiles + 3) // 4):
            nsub = min(4, Mtiles - 4 * mg)
            accs = [ps3.tile([128, T], F32, tag="mm", bufs=6, name=f"dacc{mg}_{i}") for i in range(nsub)]
            for k in range(K):
                wt = wpool.tile([128, 4, 128], BF16, tag="w512", name="wd")
                nc.sync.dma_start(
                    out=wt[:, 0:nsub, :],
                    in_=wT[k, mg].rearrange("p (j s) -> p j s", j=4)[:, 0:nsub, :],
                )
                for j in range(nsub):
                    nc.tensor.matmul(
                        accs[j][:], wt[:, j, :], rhs[:, k, :],
                        start=(k == 0), stop=(k == K - 1),
                    )
            for j in range(nsub):
                consume(4 * mg + j, accs[j])

    # proj -> x2 = xb + (proj_out + proj_b) * g1
    def proj_consume(m, acc):
        d1 = work.tile([128, T], F32, tag="d1")
        nc.vector.scalar_tensor_tensor(
            out=d1[:], in0=acc[:], scalar=mod[:, ds(m, 1)], in1=xb[:, m, :],
            op0=mybir.AluOpType.mult, op1=mybir.AluOpType.add,
        )
        nc.vector.tensor_scalar_add(
            out=x2[:, m, :], in0=d1[:], scalar1=pbg1[:, ds(m, 1)]
        )

    with nc.named_scope("proj"):
        dense(p["projwT"], CT, CT, attn, proj_consume)

    with nc.named_scope("ln2"):
        layernorm(x2, h2, 24, 40, ps3)  # s2 cols 24..31, sh2 cols 40..47

    def fc1_consume(m, acc):
        if not sim_gelu:
            nc.scalar.activation(
                out=gact[:, m, :], in_=acc[:], func=AF.Gelu_apprx_tanh,
                bias=f1b[:, ds(m, 1)],
            )
            return
        # simulator fallback: explicit tanh-approx gelu
        xs = work.tile([128, T], F32, tag="d1", name="xs")
        nc.scalar.activation(out=xs[:], in_=acc[:], func=AF.Identity,
                             bias=f1b[:, ds(m, 1)])
        t1 = work.tile([128, T], F32, tag="gsim", name="t1")
        nc.vector.tensor_tensor(out=t1[:], in0=xs[:], in1=xs[:],
                                op=mybir.AluOpType.mult)
        nc.vector.tensor_tensor(out=t1[:], in0=t1[:], in1=xs[:],
                                op=mybir.AluOpType.mult)
        nc.vector.scalar_tensor_tensor(
            out=t1[:], in0=t1[:], scalar=0.044715, in1=xs[:],
            op0=mybir.AluOpType.mult, op1=mybir.AluOpType.add,
        )
        nc.scalar.activation(out=t1[:], in_=t1[:], func=AF.Tanh,
                             scale=0.7978845608028654)
        nc.vector.tensor_scalar(
            out=t1[:], in0=t1[:], scalar1=0.5, scalar2=0.5,
            op0=mybir.AluOpType.mult, op1=mybir.AluOpType.add,
        )
        nc.vector.tensor_tensor(out=gact[:, m, :], in0=t1[:], in1=xs[:],
                                op=mybir.AluOpType.mult)

    with nc.named_scope("fc1"):
        dense(p["fc1wT"], CT, DFF // 128, h2, fc1_consume)

    def fc2_consume(m, acc):
        d1 = work.tile([128, T], F32, tag="d1")
        nc.vector.scalar_tensor_tensor(
            out=d1[:], in0=acc[:], scalar=mod[:, ds(8 + m, 1)], in1=x2[:, m, :],
            op0=mybir.AluOpType.mult, op1=mybir.AluOpType.add,
        )
        nc.vector.tensor_scalar_add(
            out=d1[:], in0=d1[:], scalar1=fbg2[:, ds(m, 1)]
        )
        nc.sync.dma_start(
            out=out_d.rearrange("(t p) q -> t p q", p=128)[m], in_=d1[:]
        )

    with nc.named_scope("fc2"):
        dense(p["fc2wT"], DFF // 128, CT, gact, fc2_consume)

    ps3.release()
    for pool in (dram, wpool, stats, work, persist, const):
        pool.release()


# --------------------------------------------------------------------------- #
# host side: shard, run, gather
# --------------------------------------------------------------------------- #

def _shard(inputs):
    bf = ml_dtypes.bfloat16
    x = np.asarray(inputs["x"], np.float32)
    cond = np.asarray(inputs["cond_BD"], np.float32)
    bias = np.asarray(inputs["attn_bias"], np.float32)[0, 0]  # [L, L]
    qkv_w = np.asarray(inputs["qkv_w"], np.float32)
    q_bias = np.asarray(inputs["q_bias"], np.float32)
    v_bias = np.asarray(inputs["v_bias"], np.float32)
    scale_mul = np.asarray(inputs["scale_mul"], np.float32).reshape(H)
    proj_w = np.asarray(inputs["proj_w"], np.float32)
    proj_b = np.asarray(inputs["proj_b"], np.float32)
    fc1_w = np.asarray(inputs["fc1_w"], np.float32)
    fc1_b = np.asarray(inputs["fc1_b"], np.float32)
    fc2_w = np.asarray(inputs["fc2_w"], np.float32)
    fc2_b = np.asarray(inputs["fc2_b"], np.float32)
    ada_w = np.asarray(inputs["ada_w"], np.float32)
    ada_b = np.asarray(inputs["ada_b"], np.float32)

    pairsel = np.zeros((2, 128), np.float32)
    pairsel[0, :64] = 1.0
    pairsel[1, 64:] = 1.0
    hsel = np.zeros((128, CT, 16), np.float32)
    for t in range(CT):
        hsel[:64, t, 2 * t] = 1.0
        hsel[64:, t, 2 * t + 1] = 1.0

    def tile_w(wT, K, MG):
        # [C_in, C_out] -> [K, MG, 128, 512] contiguous weight tiles
        return np.ascontiguousarray(
            wT.reshape(K, 128, MG, 512).transpose(0, 2, 1, 3)
        ).astype(bf)

    qkvT = qkv_w.T  # [C, 3C]
    shared = {
        "qkwT": tile_w(qkvT[:, : 2 * C], CT, 4),
        "vwT": np.ascontiguousarray(
            qkvT[:, 2 * C :].reshape(CT, 128, C)
        ).astype(bf),
        "projwT": tile_w(proj_w.T, CT, 2),
        "fc1wT": tile_w(fc1_w.T, CT, 8),
        "fc2wT": tile_w(fc2_w.T, DFF // 128, 2),

        "adab48": np.ascontiguousarray(ada_b.reshape(48, 128).T),

        "qb8": np.ascontiguousarray(q_bias.reshape(CT, 128).T),
        "vb2": v_bias.reshape(1, C).astype(bf),
        "pb8": np.ascontiguousarray(proj_b.reshape(CT, 128).T),
        "f1b": np.ascontiguousarray(fc1_b.reshape(DFF // 128, 128).T),
        "f2b": np.ascontiguousarray(fc2_b.reshape(CT, 128).T),
        "smv": scale_mul.reshape(16, 1).copy(),
        "ones128": np.ones((128, 128), np.float32).astype(bf),
        "hsel": hsel.astype(bf),
        "ones1_128": np.ones((1, 128), np.float32).astype(bf),
        "pairsel": pairsel.astype(bf),
        "ones1_64": np.ones((1, 64), np.float32).astype(bf),
        "eye48": np.eye(48, dtype=np.float32),
    }

    in_maps = []
    for core in range(NCORES):
        g, r = divmod(core, GROUP)
        qs = slice(T * r, T * (r + 1))
        m = dict(shared)
        m["xb"] = np.ascontiguousarray(
            x[g, qs].T.reshape(CT, 128, T)
        ).astype(bf)
        m["cond8"] = np.ascontiguousarray(cond[g].reshape(8, 128).T)
        m["biasT"] = np.ascontiguousarray(
            bias[qs].T.reshape(16, 128, T)
        ).astype(bf)
        aslice = ada_w.T[:, ADA_SLICE * r : ADA_SLICE * (r + 1)]
        m["adawT"] = np.ascontiguousarray(
            aslice.reshape(CT, 128, 3, 512).transpose(0, 2, 1, 3)
        ).astype(bf)


        in_maps.append(m)
    return in_maps


def kernel(**inputs):
    if "nc" not in _CACHE:
        _CACHE["nc"] = _build()
    nc = _CACHE["nc"]
    in_maps = _shard(inputs)
    try:
        res = bass_utils.run_bass_kernel_spmd(
            nc, in_maps, core_ids=list(range(NCORES))
        )
    except Exception:
        # transient device-state hiccup (seen after profiled runs); retry once
        res = bass_utils.run_bass_kernel_spmd(
            nc, in_maps, core_ids=list(range(NCORES))
        )
    out = np.empty((B, L, C), np.float32)
    for core in range(NCORES):
        g, r = divmod(core, GROUP)
        out[g, T * r : T * (r + 1)] = res.results[core]["out"].T
    return out
